# revision 6
# baseline (speedup 1.0000x reference)
"""Trainium2 Bass kernel for batched CRF forward (NeuralWordAligner _forward_alg).

Math
----
reference:  trans = einsum(trans_feats, w2) + b2;  emit = roll(emissions, 1, ax=2)
            f0 = emit[:,0,:] + log(T);  f_i = lse_k(f_{i-1}[k] + trans[j,k]) + emit_i
            alpha = lse_j(f_L)

We conjugate the state index by the roll permutation (state j' <-> emission
column j'), which removes the roll entirely:
            Mc[j,k] = trans[(j+1)%T, (k+1)%T]
            g_i[b,j] = lse_k(g_{i-1}[b,k] + Mc[j,k]) + emissions[b,i,j]
and run the scan in exp domain with a fixed per-step normalizer c:
            F_i = (exp(Mc) @ F_{i-1}) * exp(emissions_i - c)
            alpha = log(sum_j F_L[j]) + L*c + log(T)
c = log(lambda_max(exp(Mc))) + 0.5 keeps F well inside fp32 range (validated
offline: column-max logs stay within [-35, 0]; fp32 overflows at ~+88).

Layout (per core, 1024 batch elems)
-----------------------------------
State F is [113, 512] in SBUF: partitions 0-48 ("TOP") carry the 49 states of
batch group A, partitions 64-112 ("BOT") batch group B, partitions 49-63 are
zero padding (matmul output partition bases must be 32-aligned, so BOT sits at
64).  The 49x49 exp-transition matrix is duplicated block-diagonally into a
[113, 113] lhsT (zeros elsewhere), so ONE matmul with never-changing stationary
weights advances both halves: column c = 128q + r carries TOP batch 256q+r and
BOT batch 256q+128+r.

Emissions are DMA'd in natural [batch, (step,state)] layout (full-bandwidth
contiguous loads), transposed per (step, q, s) on the TensorEngine
(raw [128b, 49or64] as the stationary operand, identity streaming ->
[49or64, 128b] in PSUM at partition base 0 (TOP) / 64 (BOT)), and the
ScalarEngine applies exp(x - c) while draining PSUM -> SBUF.  TOP transposes
are 64 wide (real data from the following step lands in the pad rows) so the
pad rows of E stay finite: the chain then keeps F pad rows at exactly 0 via
the zero rows/cols of the block-diagonal lhsT.
"""

import sys

import numpy as np

if "/opt/trn_rl_repo" not in sys.path:
    sys.path.insert(0, "/opt/trn_rl_repo")

B, L, T = 8192, 48, 49
N_CORES = 8
BC = B // N_CORES          # 1024 batch per core
TP = 113                   # padded two-chain state dim: 49 + 15 pad + 49
BOT = 64                   # partition base of the second chain
NCOL = 512                 # matmul free dim (= BC // 2)
IBLK = 12                  # steps per raw-emission DMA block
NBLK = L // IBLK           # 4
RAWC = IBLK * T + 16       # 604: 16 extra cols so 64-wide transposes stay in-tile
_cache = {}


def _build_nc():
    import concourse.bass as bass
    import concourse.tile as tile
    from concourse import bacc, mybir

    dt = mybir.dt
    nc = bacc.Bacc("TRN2", target_bir_lowering=False, debug=False,
                   num_devices=N_CORES)

    em = nc.dram_tensor("em", [BC, L * T], dt.float32, kind="ExternalInput")
    lhs = nc.dram_tensor("lhs", [TP, TP], dt.float32, kind="ExternalInput")
    ones2 = nc.dram_tensor("ones2", [TP, 2], dt.float32, kind="ExternalInput")
    ident = nc.dram_tensor("ident", [128, 128], dt.float32, kind="ExternalInput")
    cbias = nc.dram_tensor("cbias", [TP, 1], dt.float32, kind="ExternalInput")
    logc = nc.dram_tensor("logc", [2, 1], dt.float32, kind="ExternalInput")
    out = nc.dram_tensor("alpha", [2, NCOL], dt.float32, kind="ExternalOutput")

    # em viewed as [q=4, il=4, p=128, s=2, c]; (q, s, p) index batch, (il, c) cols
    em_q = em.ap().rearrange("(q s p) f -> q p s f", q=4, s=2, p=128)

    EXP = mybir.ActivationFunctionType.Exp
    LOG = mybir.ActivationFunctionType.Ln

    with tile.TileContext(nc) as tc:
        with (
            tc.tile_pool(name="consts", bufs=1) as cpool,
            tc.tile_pool(name="raw", bufs=8) as rawpool,
            tc.tile_pool(name="et", bufs=4) as etpool,
            tc.tile_pool(name="fst", bufs=3) as fpool,
            tc.tile_pool(name="tail", bufs=1) as tailpool,
            tc.tile_pool(name="ptp", bufs=2, space="PSUM") as tppool,
            tc.tile_pool(name="pmm", bufs=2, space="PSUM") as mmpool,
        ):
            lhs_sb = cpool.tile([TP, TP], dt.float32, tag="lhs")
            nc.sync.dma_start(lhs_sb[:], lhs.ap())
            ones2_sb = cpool.tile([TP, 2], dt.float32, tag="ones2")
            nc.sync.dma_start(ones2_sb[:], ones2.ap())
            ident_sb = cpool.tile([128, 128], dt.float32, tag="ident")
            nc.sync.dma_start(ident_sb[:], ident.ap())
            cbias_sb = cpool.tile([TP, 1], dt.float32, tag="cbias")
            nc.sync.dma_start(cbias_sb[:], cbias.ap())
            logc_sb = cpool.tile([2, 1], dt.float32, tag="logc")
            nc.sync.dma_start(logc_sb[:], logc.ap())

            # raw emission tiles, one per (q, il): [128, (s=2) x RAWC]
            raw = {}
            for il in range(NBLK):
                ncols = RAWC if il < NBLK - 1 else IBLK * T
                for q in range(4):
                    t = rawpool.tile([128, 2 * RAWC], dt.float32, tag="raw")
                    tv = t[:].rearrange("p (s c) -> p s c", s=2)
                    src = em_q[q][:, :, il * IBLK * T: il * IBLK * T + ncols]
                    nc.sync.dma_start(tv[:, :, 0:ncols], src)
                    raw[(q, il)] = t

            f_prev = None
            for g in range(L // 2):
                # -- transposes for steps 2g, 2g+1 into one PSUM tile --
                tp = tppool.tile([TP, 2 * NCOL], dt.float32, tag="tp")
                for half in range(2):
                    i = 2 * g + half
                    il, j = divmod(i, IBLK)
                    w_top = 64 if i < L - 1 else T
                    for q in range(4):
                        rv = raw[(q, il)][:].rearrange("p (s c) -> p s c", s=2)
                        co = half * NCOL + 128 * q
                        nc.tensor.transpose(
                            tp[0:w_top, co:co + 128],
                            rv[:, 0, j * T: j * T + w_top], ident_sb[:])
                        # BOT half: plain matmul raw.T @ I == transpose,
                        # because transpose-mode outputs must start at
                        # partition 0 while this lands at base 64 (legal).
                        nc.tensor.matmul(
                            tp[BOT:BOT + T, co:co + 128],
                            rv[:, 1, j * T: j * T + T], ident_sb[:],
                            start=True, stop=True)
                # -- drain both steps: E = exp(x - c) --
                et_tile = etpool.tile([TP, 2 * NCOL], dt.float32, tag="et")
                nc.scalar.activation(et_tile[:], tp[:], EXP,
                                     bias=cbias_sb[:, 0:1], scale=1.0)

                # -- chain steps 2g, 2g+1: F_i = (M @ F_{i-1}) * E_i ; F_0 = E_0
                for half in range(2):
                    i = 2 * g + half
                    if i == 0:
                        f_prev = et_tile[:, 0:NCOL]
                        continue
                    e_slice = et_tile[:, half * NCOL:(half + 1) * NCOL]
                    p = mmpool.tile([TP, NCOL], dt.float32, tag="p")
                    nc.tensor.matmul(p[:], lhs_sb[:], f_prev,
                                     start=True, stop=True)
                    f = fpool.tile([TP, NCOL], dt.float32, tag="f")
                    if i < L - 1:
                        nc.vector.tensor_mul(f[:], p[:], e_slice)
                    else:
                        # last step: E pad rows were never written (no
                        # lookahead data) -- multiply only live partition
                        # ranges; the pad rows of this f-pool slot already
                        # hold zeros from 3 steps ago and the final ones2
                        # matmul has zero rows there.
                        nc.vector.tensor_mul(f[0:T, :], p[0:T, :],
                                             e_slice[0:T, :])
                        nc.vector.tensor_mul(f[BOT:BOT + T, :],
                                             p[BOT:BOT + T, :],
                                             e_slice[BOT:BOT + T, :])
                    f_prev = f[:]

            # alpha = log(colsum(F)) + logC
            p2 = mmpool.tile([TP, NCOL], dt.float32, tag="p")
            nc.tensor.matmul(p2[0:2, :], ones2_sb[:], f_prev, start=True, stop=True)
            la = tailpool.tile([2, NCOL], dt.float32, tag="la")
            nc.scalar.activation(la[:], p2[0:2, :], LOG)
            la2 = tailpool.tile([2, NCOL], dt.float32, tag="la2")
            nc.vector.tensor_scalar_add(la2[:], la[:], logc_sb[:, 0:1])
            nc.sync.dma_start(out.ap(), la2[:])

    nc.finalize()
    return nc


def _prep_small(trans_feats, w2, b2):
    trans = (trans_feats.astype(np.float32) @ w2.astype(np.float32)
             + b2.astype(np.float32)[0])
    mc = np.roll(np.roll(trans, -1, axis=0), -1, axis=1).astype(np.float32)
    mexp = np.exp(mc).astype(np.float32)
    lam = float(np.abs(np.linalg.eigvals(mexp.astype(np.float64))).max())
    c_step = float(np.log(lam) + 0.5)

    lhs = np.zeros((TP, TP), dtype=np.float32)
    lhs[0:T, 0:T] = mexp.T          # lhsT[k, j] = Mexp[j, k]
    lhs[BOT:BOT + T, BOT:BOT + T] = mexp.T
    ones2 = np.zeros((TP, 2), dtype=np.float32)
    ones2[0:T, 0] = 1.0
    ones2[BOT:BOT + T, 1] = 1.0
    ident = np.eye(128, dtype=np.float32)
    cbias = np.full((TP, 1), -c_step, dtype=np.float32)
    logc = np.full((2, 1), L * c_step + np.log(T), dtype=np.float32)
    return lhs, ones2, ident, cbias, logc


def _make_in_maps(emissions, trans_feats, w2, b2):
    lhs, ones2, ident, cbias, logc = _prep_small(trans_feats, w2, b2)
    em = np.ascontiguousarray(emissions.astype(np.float32)).reshape(B, L * T)
    return [{
        "em": em[c * BC:(c + 1) * BC],
        "lhs": lhs, "ones2": ones2, "ident": ident,
        "cbias": cbias, "logc": logc,
    } for c in range(N_CORES)]


def kernel(emissions, trans_feats, w2, b2):
    from concourse.bass_utils import run_bass_kernel_spmd

    emissions = np.asarray(emissions)
    trans_feats = np.asarray(trans_feats)
    w2 = np.asarray(w2)
    b2 = np.asarray(b2)

    if "nc" not in _cache:
        _cache["nc"] = _build_nc()
    nc = _cache["nc"]

    in_maps = _make_in_maps(emissions, trans_feats, w2, b2)
    res = run_bass_kernel_spmd(nc, in_maps, core_ids=list(range(N_CORES)))

    alpha = np.empty(B, dtype=np.float32)
    for c in range(N_CORES):
        o = res.results[c]["alpha"]          # [2, 512]
        # col = 128q + r, slot s -> local batch 256q + 128s + r
        alpha[c * BC:(c + 1) * BC] = (
            o.reshape(2, 4, 128).transpose(1, 0, 2).reshape(BC)
        )
    return alpha


# revision 7
# speedup vs baseline: 1.5433x; 1.5433x over previous
"""Trainium2 Bass kernel for batched CRF forward (NeuralWordAligner _forward_alg).

Math
----
reference:  trans = einsum(trans_feats, w2) + b2;  emit = roll(emissions, 1, ax=2)
            f0 = emit[:,0,:] + log(T);  f_i = lse_k(f_{i-1}[k] + trans[j,k]) + emit_i
            alpha = lse_j(f_L)

We conjugate the state index by the roll permutation (state j' <-> emission
column j'), which removes the roll entirely:
            Mc[j,k] = trans[(j+1)%T, (k+1)%T]
            g_i[b,j] = lse_k(g_{i-1}[b,k] + Mc[j,k]) + emissions[b,i,j]
and run the scan in exp domain with a fixed per-step normalizer c:
            F_i = (exp(Mc) @ F_{i-1}) * exp(emissions_i - c)
            alpha = log(sum_j F_L[j]) + L*c + log(T)
c = log(lambda_max(exp(Mc))) + 0.5 keeps F well inside fp32 range (validated
offline: column-max logs stay within [-35, 0]; fp32 overflows at ~+88).

Layout (per core, 1024 batch elems)
-----------------------------------
State F is [113, 512] in SBUF: partitions 0-48 ("TOP") carry the 49 states of
batch group A, partitions 64-112 ("BOT") batch group B, partitions 49-63 are
zero padding (matmul output partition bases must be 32-aligned, so BOT sits at
64).  The 49x49 exp-transition matrix is duplicated block-diagonally into a
[113, 113] lhsT (zeros elsewhere), so ONE matmul with never-changing stationary
weights advances both halves: column c = 128q + r carries TOP batch 256q+r and
BOT batch 256q+128+r.

Emissions are DMA'd in natural [batch, (step,state)] layout (full-bandwidth
contiguous loads), transposed per (step, q, s) on the TensorEngine
(raw [128b, 49or64] as the stationary operand, identity streaming ->
[49or64, 128b] in PSUM at partition base 0 (TOP) / 64 (BOT)), and the
ScalarEngine applies exp(x - c) while draining PSUM -> SBUF.  TOP transposes
are 64 wide (real data from the following step lands in the pad rows) so the
pad rows of E stay finite: the chain then keeps F pad rows at exactly 0 via
the zero rows/cols of the block-diagonal lhsT.
"""

import sys

import numpy as np

if "/opt/trn_rl_repo" not in sys.path:
    sys.path.insert(0, "/opt/trn_rl_repo")

B, L, T = 8192, 48, 49
N_CORES = 8
BC = B // N_CORES          # 1024 batch per core
TP = 113                   # padded two-chain state dim: 49 + 15 pad + 49
BOT = 64                   # partition base of the second chain
NCOL = 512                 # matmul free dim (= BC // 2)
IBLK = 12                  # steps per raw-emission DMA block
NBLK = L // IBLK           # 4
RAWC = IBLK * T + 16       # 604: 16 extra cols so 64-wide transposes stay in-tile
_cache = {}


def _build_nc():
    import concourse.bass as bass
    import concourse.tile as tile
    from concourse import bacc, mybir

    dt = mybir.dt
    nc = bacc.Bacc("TRN2", target_bir_lowering=False, debug=False,
                   num_devices=N_CORES)

    em = nc.dram_tensor("em", [BC, L * T], dt.float32, kind="ExternalInput")
    lhs = nc.dram_tensor("lhs", [TP, TP], dt.bfloat16, kind="ExternalInput")
    ones2 = nc.dram_tensor("ones2", [TP, 2], dt.bfloat16, kind="ExternalInput")
    ident = nc.dram_tensor("ident", [128, 128], dt.bfloat16, kind="ExternalInput")
    cbias = nc.dram_tensor("cbias", [TP, 1], dt.float32, kind="ExternalInput")
    logc = nc.dram_tensor("logc", [2, 1], dt.float32, kind="ExternalInput")
    out = nc.dram_tensor("alpha", [2, NCOL], dt.float32, kind="ExternalOutput")

    # em viewed as [q=4, il=4, p=128, s=2, c]; (q, s, p) index batch, (il, c) cols
    em_q = em.ap().rearrange("(q s p) f -> q p s f", q=4, s=2, p=128)

    EXP = mybir.ActivationFunctionType.Exp
    LOG = mybir.ActivationFunctionType.Ln

    with tile.TileContext(nc) as tc:
        with (
            tc.tile_pool(name="consts", bufs=1) as cpool,
            tc.tile_pool(name="raw", bufs=16) as rawpool,
            tc.tile_pool(name="et", bufs=4) as etpool,
            tc.tile_pool(name="fst", bufs=3) as fpool,
            tc.tile_pool(name="tail", bufs=1) as tailpool,
            tc.tile_pool(name="ptp", bufs=2, space="PSUM") as tppool,
            tc.tile_pool(name="pmm", bufs=3, space="PSUM") as mmpool,
        ):
            lhs_sb = cpool.tile([TP, TP], dt.bfloat16, tag="lhs")
            nc.sync.dma_start(lhs_sb[:], lhs.ap())
            ones2_sb = cpool.tile([TP, 2], dt.bfloat16, tag="ones2")
            nc.sync.dma_start(ones2_sb[:], ones2.ap())
            ident_sb = cpool.tile([128, 128], dt.bfloat16, tag="ident")
            nc.sync.dma_start(ident_sb[:], ident.ap())
            cbias_sb = cpool.tile([TP, 1], dt.float32, tag="cbias")
            nc.sync.dma_start(cbias_sb[:], cbias.ap())
            logc_sb = cpool.tile([2, 1], dt.float32, tag="logc")
            nc.sync.dma_start(logc_sb[:], logc.ap())

            # raw emission tiles, one per (q, il): [128, (s=2) x RAWC]
            raw = {}
            for il in range(NBLK):
                ncols = RAWC if il < NBLK - 1 else IBLK * T
                for q in range(4):
                    t = rawpool.tile([128, 2 * RAWC], dt.bfloat16, tag="raw")
                    tv = t[:].rearrange("p (s c) -> p s c", s=2)
                    src = em_q[q][:, :, il * IBLK * T: il * IBLK * T + ncols]
                    nc.gpsimd.dma_start(tv[:, :, 0:ncols], src)  # fp32 -> bf16 cast
                    raw[(q, il)] = t

            f_prev = None
            for g in range(L // 2):
                # -- transposes for steps 2g, 2g+1 into one PSUM tile --
                tp = tppool.tile([TP, 2 * NCOL], dt.float32, tag="tp")
                for half in range(2):
                    i = 2 * g + half
                    il, j = divmod(i, IBLK)
                    w_top = 64 if i < L - 1 else T
                    for q in range(4):
                        rv = raw[(q, il)][:].rearrange("p (s c) -> p s c", s=2)
                        co = half * NCOL + 128 * q
                        nc.tensor.matmul(
                            tp[0:w_top, co:co + 128],
                            rv[:, 0, j * T: j * T + w_top], ident_sb[:],
                            start=True, stop=True)
                        # BOT half: plain matmul raw.T @ I == transpose,
                        # because transpose-mode outputs must start at
                        # partition 0 while this lands at base 64 (legal).
                        nc.tensor.matmul(
                            tp[BOT:BOT + T, co:co + 128],
                            rv[:, 1, j * T: j * T + T], ident_sb[:],
                            start=True, stop=True)
                # -- drain both steps: E = exp(x - c) --
                et_tile = etpool.tile([TP, 2 * NCOL], dt.float32, tag="et")
                nc.scalar.activation(et_tile[:], tp[:], EXP,
                                     bias=cbias_sb[:, 0:1], scale=1.0)

                # -- chain steps 2g, 2g+1: F_i = (M @ F_{i-1}) * E_i ; F_0 = E_0
                for half in range(2):
                    i = 2 * g + half
                    if i == 0:
                        f0 = fpool.tile([TP, NCOL], dt.bfloat16, tag="f")
                        nc.vector.tensor_copy(f0[:], et_tile[:, 0:NCOL])
                        f_prev = f0[:]
                        continue
                    e_slice = et_tile[:, half * NCOL:(half + 1) * NCOL]
                    p = mmpool.tile([TP, NCOL], dt.float32, tag="p")
                    nc.tensor.matmul(p[:], lhs_sb[:], f_prev,
                                     start=True, stop=True)
                    f = fpool.tile([TP, NCOL], dt.bfloat16, tag="f")
                    if i < L - 1:
                        nc.vector.tensor_mul(f[:], p[:], e_slice)
                    else:
                        # last step: E pad rows were never written (no
                        # lookahead data) -- multiply only live partition
                        # ranges; the pad rows of this f-pool slot already
                        # hold zeros from 3 steps ago and the final ones2
                        # matmul has zero rows there.
                        nc.vector.tensor_mul(f[0:T, :], p[0:T, :],
                                             e_slice[0:T, :])
                        nc.vector.tensor_mul(f[BOT:BOT + T, :],
                                             p[BOT:BOT + T, :],
                                             e_slice[BOT:BOT + T, :])
                    f_prev = f[:]

            # alpha = log(colsum(F)) + logC
            p2 = mmpool.tile([TP, NCOL], dt.float32, tag="p")
            nc.tensor.matmul(p2[0:2, :], ones2_sb[:], f_prev, start=True, stop=True)
            la = tailpool.tile([2, NCOL], dt.float32, tag="la")
            nc.scalar.activation(la[:], p2[0:2, :], LOG)
            la2 = tailpool.tile([2, NCOL], dt.float32, tag="la2")
            nc.vector.tensor_scalar_add(la2[:], la[:], logc_sb[:, 0:1])
            nc.sync.dma_start(out.ap(), la2[:])

    nc.finalize()
    return nc


def _prep_small(trans_feats, w2, b2):
    trans = (trans_feats.astype(np.float32) @ w2.astype(np.float32)
             + b2.astype(np.float32)[0])
    mc = np.roll(np.roll(trans, -1, axis=0), -1, axis=1).astype(np.float32)
    mexp = np.exp(mc).astype(np.float32)
    lam = float(np.abs(np.linalg.eigvals(mexp.astype(np.float64))).max())
    c_step = float(np.log(lam) + 0.5)

    import ml_dtypes
    lhs = np.zeros((TP, TP), dtype=ml_dtypes.bfloat16)
    lhs[0:T, 0:T] = mexp.T          # lhsT[k, j] = Mexp[j, k]
    lhs[BOT:BOT + T, BOT:BOT + T] = mexp.T
    ones2 = np.zeros((TP, 2), dtype=ml_dtypes.bfloat16)
    ones2[0:T, 0] = 1.0
    ones2[BOT:BOT + T, 1] = 1.0
    ident = np.eye(128, dtype=np.float32).astype(ml_dtypes.bfloat16)
    cbias = np.full((TP, 1), -c_step, dtype=np.float32)
    logc = np.full((2, 1), L * c_step + np.log(T), dtype=np.float32)
    return lhs, ones2, ident, cbias, logc


def _make_in_maps(emissions, trans_feats, w2, b2):
    lhs, ones2, ident, cbias, logc = _prep_small(trans_feats, w2, b2)
    em = np.ascontiguousarray(emissions.astype(np.float32)).reshape(B, L * T)
    return [{
        "em": em[c * BC:(c + 1) * BC],
        "lhs": lhs, "ones2": ones2, "ident": ident,
        "cbias": cbias, "logc": logc,
    } for c in range(N_CORES)]


def kernel(emissions, trans_feats, w2, b2):
    from concourse.bass_utils import run_bass_kernel_spmd

    emissions = np.asarray(emissions)
    trans_feats = np.asarray(trans_feats)
    w2 = np.asarray(w2)
    b2 = np.asarray(b2)

    if "nc" not in _cache:
        _cache["nc"] = _build_nc()
    nc = _cache["nc"]

    in_maps = _make_in_maps(emissions, trans_feats, w2, b2)
    res = run_bass_kernel_spmd(nc, in_maps, core_ids=list(range(N_CORES)))

    alpha = np.empty(B, dtype=np.float32)
    for c in range(N_CORES):
        o = res.results[c]["alpha"]          # [2, 512]
        # col = 128q + r, slot s -> local batch 256q + 128s + r
        alpha[c * BC:(c + 1) * BC] = (
            o.reshape(2, 4, 128).transpose(1, 0, 2).reshape(BC)
        )
    return alpha
